# revision 2
# baseline (speedup 1.0000x reference)
"""CrossNet forward as a Trainium2 Bass/Tile kernel, data-parallel over 8 cores.

Math: the CrossNet layer stack
    x_{l+1} = x0 * (x_l . w_l) + b_l + x_l            (l = 0..3)
collapses in closed form.  Writing x_l = x0 * alpha_l[b] + beta_l[d]:
    p_l[b]     = sum_d x0[b,d] w_l[d]                 (4 projections of x0)
    alpha_0    = 1,   alpha_{l+1} = alpha_l * (1 + p_l) + c_l
    beta_{l+1} = beta_l + b_l,  c_l = beta_l . w_l    (host-computable scalars)
    out        = x0 * alpha_4[b] + beta_4[d]

Device work per 128-row chunk: PE transpose of the chunk, one [128d,128b]^T @
[128d,4] matmul for the projections, a tiny DVE recurrence for alpha, and one
tensor_scalar multiply for the output.  Everything streams: the kernel is
HBM-bound (~64 MB per core through ~358 GB/s).
"""

import os

import numpy as np

B = 500_000
D = 128
L = 4
N_CORES = 8
ROWS = B // N_CORES          # 62500 rows per core
G = 8                        # 128-row chunks per supertile
SUP = 128 * G                # 1024 rows per supertile
NSUP = ROWS // SUP           # 61 full supertiles
REM = ROWS - NSUP * SUP      # 36 remainder rows

_CACHE: dict = {}

# test.py can read run metadata (exec_time_ns etc.) from here after a call.
LAST_RESULTS = None


def _build(cs, has_bias):
    import concourse.tile as tile
    from concourse import bacc, mybir

    f32 = mybir.dt.float32
    mult = mybir.AluOpType.mult
    add = mybir.AluOpType.add

    nc = bacc.Bacc(
        "TRN2",
        target_bir_lowering=False,
        debug=False,
        enable_asserts=False,
        num_devices=N_CORES,
    )
    x = nc.dram_tensor("x", [ROWS, D], f32, kind="ExternalInput").ap()
    w = nc.dram_tensor("w", [D, L], f32, kind="ExternalInput").ap()
    ident = nc.dram_tensor("ident", [128, 128], f32, kind="ExternalInput").ap()
    bb = None
    if has_bias:
        bb = nc.dram_tensor("bb", [128, D], f32, kind="ExternalInput").ap()
    out = nc.dram_tensor("out", [ROWS, D], f32, kind="ExternalOutput").ap()

    # Supertile views: partition p <-> 8 consecutive HBM rows, so each
    # partition's free dim (g d) is one contiguous 4 KB run.
    xv = x[0 : NSUP * SUP, :].rearrange("(s p g) d -> s p (g d)", p=128, g=G)
    ov = out[0 : NSUP * SUP, :].rearrange("(s p g) d -> s p (g d)", p=128, g=G)

    with tile.TileContext(nc) as tc:
        with (
            tc.tile_pool(name="consts", bufs=1) as cpool,
            tc.tile_pool(name="xin", bufs=4) as xpool,
            tc.tile_pool(name="xt", bufs=3) as xtpool,
            tc.tile_pool(name="xtps", bufs=2, space="PSUM") as tps_pool,
            tc.tile_pool(name="ptps", bufs=2, space="PSUM") as pps_pool,
            tc.tile_pool(name="small", bufs=4) as spool,
            tc.tile_pool(name="outp", bufs=4) as opool,
        ):
            ident_sb = cpool.tile([128, 128], f32, tag="ident")
            nc.sync.dma_start(ident_sb[:], ident)
            w_sb = cpool.tile([D, L], f32, tag="w")
            nc.sync.dma_start(w_sb[:], w)
            bb_sb = None
            if has_bias:
                bb_sb = cpool.tile([128, D], f32, tag="bb")
                nc.sync.dma_start(bb_sb[:], bb)

            def block(in_ap, out_ap, p_cnt, g_cnt):
                # in_ap/out_ap: [p_cnt, g_cnt*128] DRAM views; chunk g holds
                # 128 feature columns of p_cnt independent rows.
                x_sb = xpool.tile([p_cnt, g_cnt * D], f32, tag="x")
                nc.sync.dma_start(x_sb[:], in_ap)

                xt_ps = tps_pool.tile([128, g_cnt * p_cnt], f32, tag="xtps")
                for g in range(g_cnt):
                    nc.tensor.transpose(
                        xt_ps[:, g * p_cnt : (g + 1) * p_cnt],
                        x_sb[:, g * D : (g + 1) * D],
                        ident_sb[:p_cnt, :p_cnt],
                    )
                xt_sb = xtpool.tile([128, g_cnt * p_cnt], f32, tag="xt")
                ncols = g_cnt * p_cnt
                half = (ncols + 1) // 2
                nc.scalar.copy(xt_sb[:, :half], xt_ps[:, :half])
                if ncols > half:
                    nc.scalar.copy(xt_sb[:, half:ncols], xt_ps[:, half:ncols])

                pt_ps = pps_pool.tile([p_cnt, L * g_cnt], f32, tag="pt")
                for g in range(g_cnt):
                    nc.tensor.matmul(
                        pt_ps[:, g * L : (g + 1) * L],
                        lhsT=xt_sb[:, g * p_cnt : (g + 1) * p_cnt],
                        rhs=w_sb[:],
                        start=True,
                        stop=True,
                    )

                # q = 1 + p, then alpha = Horner chain over the 4 layers.
                q_sb = spool.tile([p_cnt, L * g_cnt], f32, tag="q")
                nc.vector.tensor_scalar_add(q_sb[:], pt_ps[:], 1.0)
                qv = q_sb[:].rearrange("p (g l) -> p g l", l=L)
                if has_bias:
                    a = spool.tile([p_cnt, g_cnt], f32, tag="a0")
                    # c_0 == 0 always (beta_0 = 0)
                    nc.vector.tensor_copy(a[:], qv[:, :, 0])
                    for l in range(1, L):
                        t = spool.tile([p_cnt, g_cnt], f32, tag=f"a{l}")
                        nc.vector.tensor_mul(t[:], a[:], qv[:, :, l])
                        if cs[l] != 0.0:
                            t2 = spool.tile([p_cnt, g_cnt], f32, tag=f"ac{l}")
                            nc.vector.tensor_scalar_add(t2[:], t[:], float(cs[l]))
                            t = t2
                        a = t
                else:
                    a1 = spool.tile([p_cnt, g_cnt], f32, tag="a1")
                    nc.vector.tensor_mul(a1[:], qv[:, :, 0], qv[:, :, 1])
                    a2 = spool.tile([p_cnt, g_cnt], f32, tag="a2")
                    nc.vector.tensor_mul(a2[:], a1[:], qv[:, :, 2])
                    a = spool.tile([p_cnt, g_cnt], f32, tag="a3")
                    nc.vector.tensor_mul(a[:], a2[:], qv[:, :, 3])

                out_sb = opool.tile([p_cnt, g_cnt * D], f32, tag="o")
                for g in range(g_cnt):
                    if has_bias:
                        nc.vector.scalar_tensor_tensor(
                            out_sb[:, g * D : (g + 1) * D],
                            x_sb[:, g * D : (g + 1) * D],
                            a[:, g : g + 1],
                            bb_sb[:p_cnt, :],
                            op0=mult,
                            op1=add,
                        )
                    else:
                        nc.vector.tensor_scalar_mul(
                            out_sb[:, g * D : (g + 1) * D],
                            x_sb[:, g * D : (g + 1) * D],
                            a[:, g : g + 1],
                        )
                nc.sync.dma_start(out_ap, out_sb[:])

            for s in range(NSUP):
                block(xv[s], ov[s], 128, G)
            if REM:
                block(x[NSUP * SUP :, :], out[NSUP * SUP :, :], REM, 1)

    nc.compile()
    return nc


def kernel(inputs, kernels, biases):
    global LAST_RESULTS
    from concourse.bass_utils import run_bass_kernel_spmd

    x = np.ascontiguousarray(np.asarray(inputs), dtype=np.float32)
    assert x.shape == (B, D), x.shape
    kern = np.asarray(kernels, dtype=np.float32).reshape(L, D)
    bias = np.asarray(biases, dtype=np.float32).reshape(L, D)

    W = np.ascontiguousarray(kern.T)  # [D, L]
    has_bias = bool(np.any(bias))
    cs = []
    beta = np.zeros(D, dtype=np.float32)
    for l in range(L):
        cs.append(float(np.dot(beta.astype(np.float64), kern[l].astype(np.float64))))
        beta = beta + bias[l]

    key = (has_bias, tuple(cs) if has_bias else None)
    nc = _CACHE.get(key)
    if nc is None:
        nc = _build(cs, has_bias)
        _CACHE[key] = nc

    ident = np.eye(128, dtype=np.float32)
    bbcast = np.ascontiguousarray(np.broadcast_to(beta, (128, D)), dtype=np.float32)
    in_maps = []
    for i in range(N_CORES):
        m = {"x": x[i * ROWS : (i + 1) * ROWS], "w": W, "ident": ident}
        if has_bias:
            m["bb"] = bbcast
        in_maps.append(m)

    res = run_bass_kernel_spmd(nc, in_maps, core_ids=list(range(N_CORES)))
    LAST_RESULTS = res
    return np.concatenate([res.results[i]["out"] for i in range(N_CORES)], axis=0)


# revision 8
# speedup vs baseline: 1.2106x; 1.2106x over previous
"""CrossNet forward as a Trainium2 Bass/Tile kernel, data-parallel over 8 cores.

Math: the CrossNet layer stack
    x_{l+1} = x0 * (x_l . w_l) + b_l + x_l            (l = 0..3)
collapses in closed form.  Writing x_l = x0 * alpha_l[b] + beta_l[d]:
    p_l[b]     = sum_d x0[b,d] w_l[d]                 (4 projections of x0)
    alpha_0    = 1,   alpha_{l+1} = alpha_l * (1 + p_l) + c_l
    beta_{l+1} = beta_l + b_l,  c_l = beta_l . w_l    (host-computable scalars)
    out        = x0 * alpha_4[b] + beta_4[d]

Device work per 128-row chunk: PE transpose of the chunk, one [128d,128b]^T @
[128d,4] matmul for the projections, a tiny DVE recurrence for alpha, and one
broadcast multiply for the output.  Everything streams: the kernel is
HBM-bound (~64 MB per core through ~358 GB/s).
"""

import numpy as np

B = 500_000
D = 128
L = 4
N_CORES = 8
ROWS = B // N_CORES          # 62500 rows per core
G = 8                        # 128-row chunks per supertile
SUP = 128 * G                # 1024 rows per supertile
NSUP = ROWS // SUP           # 61 full supertiles
REM = ROWS - NSUP * SUP      # 36 remainder rows

# float32r P-matmul: streams 1-pass (vs fp32's LOW_HIGH 2-pass) and halves the
# weight-load cost. The walrus verifier requires f32r matmul operands to be
# produced by a rounding instruction, so the PSUM->SBUF ACT copy of the
# transposed tile emits f32r (free rounding), and w is rounded once on-device.
# Precision on HW is validated by test_precision.py.
F32R_P = True

_CACHE: dict = {}

# test.py can read run metadata (exec_time_ns etc.) from here after a call.
LAST_RESULTS = None


def _build(cs, has_bias):
    import concourse.tile as tile
    from concourse import bacc, mybir

    f32 = mybir.dt.float32
    pdt = mybir.dt.float32r if F32R_P else f32
    mult = mybir.AluOpType.mult
    add = mybir.AluOpType.add

    nc = bacc.Bacc(
        "TRN2",
        target_bir_lowering=False,
        debug=False,
        enable_asserts=False,
        num_devices=N_CORES,
    )
    x = nc.dram_tensor("x", [ROWS, D], f32, kind="ExternalInput").ap()
    w = nc.dram_tensor("w", [D, L], f32, kind="ExternalInput").ap()
    ident = nc.dram_tensor("ident", [128, 128], f32, kind="ExternalInput").ap()
    bb = None
    if has_bias:
        bb = nc.dram_tensor("bb", [128, D], f32, kind="ExternalInput").ap()
    out = nc.dram_tensor("out", [ROWS, D], f32, kind="ExternalOutput").ap()

    # Supertile views: partition p <-> 8 consecutive HBM rows, so each
    # partition's free dim (g d) is one contiguous 4 KB run.
    xv = x[0 : NSUP * SUP, :].rearrange("(s p g) d -> s p (g d)", p=128, g=G)
    ov = out[0 : NSUP * SUP, :].rearrange("(s p g) d -> s p (g d)", p=128, g=G)

    with tile.TileContext(nc) as tc:
        with (
            tc.tile_pool(name="consts", bufs=1) as cpool,
            tc.tile_pool(name="xin", bufs=4) as xpool,
            tc.tile_pool(name="xt", bufs=3) as xtpool,
            tc.tile_pool(name="xtps", bufs=2, space="PSUM") as tps_pool,
            tc.tile_pool(name="ptps", bufs=2, space="PSUM") as pps_pool,
            tc.tile_pool(name="small", bufs=4) as spool,
            tc.tile_pool(name="outp", bufs=4) as opool,
        ):
            ident_sb = cpool.tile([128, 128], f32, tag="ident")
            nc.sync.dma_start(ident_sb[:], ident)
            w_in = cpool.tile([D, L], f32, tag="w_in")
            nc.sync.dma_start(w_in[:], w)
            w_sb = w_in
            if F32R_P:
                # Round w to f32r once so the P matmuls accept it.
                w_sb = cpool.tile([D, L], pdt, tag="w_r")
                nc.scalar.copy(w_sb[:], w_in[:])
            bb_sb = None
            if has_bias:
                bb_sb = cpool.tile([128, D], f32, tag="bb")
                nc.sync.dma_start(bb_sb[:], bb)

            def block(in_ap, out_ap, p_cnt, g_cnt):
                # in_ap/out_ap: [p_cnt, g_cnt*128] DRAM views; chunk g holds
                # 128 feature columns of p_cnt independent rows.
                x_sb = xpool.tile([p_cnt, g_cnt * D], f32, tag="x")
                nc.sync.dma_start(x_sb[:], in_ap)

                xt_ps = tps_pool.tile([128, g_cnt * p_cnt], f32, tag="xtps")
                xt_sb = xtpool.tile([128, g_cnt * p_cnt], pdt, tag="xt")
                pt_ps = pps_pool.tile([p_cnt, L * g_cnt], f32, tag="pt")

                ncols = g_cnt * p_cnt
                half_g = (g_cnt + 1) // 2

                def emit_transpose(g):
                    nc.tensor.transpose(
                        xt_ps[:, g * p_cnt : (g + 1) * p_cnt],
                        x_sb[:, g * D : (g + 1) * D],
                        ident_sb[:p_cnt, :p_cnt],
                    )

                def emit_copy(c0, c1):
                    nc.scalar.copy(xt_sb[:, c0:c1], xt_ps[:, c0:c1])

                def emit_p(g):
                    nc.tensor.matmul(
                        pt_ps[:, g * L : (g + 1) * L],
                        lhsT=xt_sb[:, g * p_cnt : (g + 1) * p_cnt],
                        rhs=w_sb[:],
                        start=True,
                        stop=True,
                    )

                for g in range(half_g):
                    emit_transpose(g)
                emit_copy(0, half_g * p_cnt)
                for g in range(half_g, g_cnt):
                    emit_transpose(g)
                if g_cnt > half_g:
                    emit_copy(half_g * p_cnt, ncols)
                for g in range(g_cnt):
                    emit_p(g)

                # q = 1 + p, then alpha = Horner chain over the 4 layers.
                q_sb = spool.tile([p_cnt, L * g_cnt], f32, tag="q")
                nc.vector.tensor_scalar_add(q_sb[:], pt_ps[:], 1.0)
                qv = q_sb[:].rearrange("p (g l) -> p g l", l=L)
                if has_bias:
                    a = spool.tile([p_cnt, g_cnt], f32, tag="a0")
                    # c_0 == 0 always (beta_0 = 0)
                    nc.vector.tensor_copy(a[:], qv[:, :, 0])
                    for l in range(1, L):
                        t = spool.tile([p_cnt, g_cnt], f32, tag=f"a{l}")
                        nc.vector.tensor_mul(t[:], a[:], qv[:, :, l])
                        if cs[l] != 0.0:
                            t2 = spool.tile([p_cnt, g_cnt], f32, tag=f"ac{l}")
                            nc.vector.tensor_scalar_add(t2[:], t[:], float(cs[l]))
                            t = t2
                        a = t
                else:
                    a1 = spool.tile([p_cnt, g_cnt], f32, tag="a1")
                    nc.vector.tensor_mul(a1[:], qv[:, :, 0], qv[:, :, 1])
                    a2 = spool.tile([p_cnt, g_cnt], f32, tag="a2")
                    nc.vector.tensor_mul(a2[:], a1[:], qv[:, :, 2])
                    a = spool.tile([p_cnt, g_cnt], f32, tag="a3")
                    nc.vector.tensor_mul(a[:], a2[:], qv[:, :, 3])

                out_sb = opool.tile([p_cnt, g_cnt * D], f32, tag="o")
                if has_bias:
                    for g in range(g_cnt):
                        nc.vector.scalar_tensor_tensor(
                            out_sb[:, g * D : (g + 1) * D],
                            x_sb[:, g * D : (g + 1) * D],
                            a[:, g : g + 1],
                            bb_sb[:p_cnt, :],
                            op0=mult,
                            op1=add,
                        )
                else:
                    # out[p, g, d] = x[p, g, d] * a[p, g]: one broadcast TT per
                    # engine half (DVE + GpSimd run concurrently).
                    xv3 = x_sb[:].rearrange("p (g d) -> p g d", d=D)
                    ov3 = out_sb[:].rearrange("p (g d) -> p g d", d=D)
                    g_dve = g_cnt if g_cnt == 1 else half_g
                    nc.vector.tensor_mul(
                        ov3[:, 0:g_dve, :],
                        xv3[:, 0:g_dve, :],
                        a[:, 0:g_dve].to_broadcast([p_cnt, g_dve, D]),
                    )
                    if g_cnt > g_dve:
                        nc.gpsimd.tensor_mul(
                            ov3[:, g_dve:g_cnt, :],
                            xv3[:, g_dve:g_cnt, :],
                            a[:, g_dve:g_cnt].to_broadcast(
                                [p_cnt, g_cnt - g_dve, D]
                            ),
                        )
                nc.sync.dma_start(out_ap, out_sb[:])

            for s in range(NSUP):
                block(xv[s], ov[s], 128, G)
            if REM:
                block(x[NSUP * SUP :, :], out[NSUP * SUP :, :], REM, 1)

    nc.compile()
    return nc


def kernel(inputs, kernels, biases):
    global LAST_RESULTS
    from concourse.bass_utils import run_bass_kernel_spmd

    x = np.ascontiguousarray(np.asarray(inputs), dtype=np.float32)
    assert x.shape == (B, D), x.shape
    kern = np.asarray(kernels, dtype=np.float32).reshape(L, D)
    bias = np.asarray(biases, dtype=np.float32).reshape(L, D)

    W = np.ascontiguousarray(kern.T)  # [D, L]
    has_bias = bool(np.any(bias))
    cs = []
    beta = np.zeros(D, dtype=np.float32)
    for l in range(L):
        cs.append(float(np.dot(beta.astype(np.float64), kern[l].astype(np.float64))))
        beta = beta + bias[l]

    key = (has_bias, tuple(cs) if has_bias else None)
    nc = _CACHE.get(key)
    if nc is None:
        nc = _build(cs, has_bias)
        _CACHE[key] = nc

    ident = np.eye(128, dtype=np.float32)
    bbcast = np.ascontiguousarray(np.broadcast_to(beta, (128, D)), dtype=np.float32)
    in_maps = []
    for i in range(N_CORES):
        m = {"x": x[i * ROWS : (i + 1) * ROWS], "w": W, "ident": ident}
        if has_bias:
            m["bb"] = bbcast
        in_maps.append(m)

    res = run_bass_kernel_spmd(nc, in_maps, core_ids=list(range(N_CORES)))
    LAST_RESULTS = res
    return np.concatenate([res.results[i]["out"] for i in range(N_CORES)], axis=0)
